# revision 4
# baseline (speedup 1.0000x reference)
"""Distributed Bass kernel for nn_Attention (dense transformer block with the
q=k=v source bug) on 8 TRN2 NeuronCores.

Sharding: tensor-parallel over heads (Megatron-style). Core i owns heads
(2i, 2i+1):
  1. k-projection: KT[d, t] = (x @ W_k_slice).T computed from host-transposed
     x^T so no on-device transpose of x is needed. Only the k third of W_attn
     matters (reference takes q = k = v from the k slice).
  2. Scores S = K K^T are SYMMETRIC (q == k), so tiles are produced in
     [key-block, query] layout directly; exp's accum_out gives the softmax
     denominators (row sums == column sums) for free.
  3. O^T = K^T @ E accumulated in PSUM (col-packed: 2 heads side by side).
  4. Normalize by 1/s(q), AllToAll so core i ends with all 1024 head-dims for
     its 256-token block per batch, then the output projection + bias.
Host gathers the 8 per-core [2, 256, 1024] row blocks into [2, 2048, 1024].
"""

import numpy as np

import concourse.bass as bass
import concourse.tile as tile
from concourse import bacc, mybir
from concourse.bass_utils import run_bass_kernel_spmd
from concourse.masks import make_identity

N_CORES = 8
B, L, D = 2, 2048, 1024
H, HD = 16, 64
HPC = H // N_CORES  # heads per core = 2
DC = HPC * HD  # head-dim columns per core = 128
TPB = L // N_CORES  # tokens per batch per core (proj stage) = 256
F32 = mybir.dt.float32
F32R = mybir.dt.float32r
BF16 = mybir.dt.bfloat16
QS = 1024  # query span per S-tile (PSUM budget: 2 banks)
KB = L // 128  # 16 key blocks per batch


def _r(ap):
    return ap.bitcast(F32R)


def build():
    nc = bacc.Bacc("TRN2", target_bir_lowering=False, debug=False, num_devices=N_CORES)
    xt = nc.dram_tensor("xt", [B, D, L], BF16, kind="ExternalInput")
    wk = nc.dram_tensor("wk", [D, DC], BF16, kind="ExternalInput")
    bk = nc.dram_tensor("bk", [DC, 1], F32, kind="ExternalInput")
    wp = nc.dram_tensor("wp", [D, D], BF16, kind="ExternalInput")
    bp = nc.dram_tensor("bp", [D], F32, kind="ExternalInput")
    out = nc.dram_tensor("out", [B, TPB, D], F32, kind="ExternalOutput")

    xt_v = xt.ap().rearrange("b (dc p) t -> b dc p t", p=128)  # [B, 8, 128, L]
    wk_v = wk.ap().rearrange("(dc p) m -> p dc m", p=128)  # [128, 8, DC]
    wp_v = wp.ap().rearrange("(dc p) n -> p dc n", p=128)  # [128, 8, D]

    with tile.TileContext(nc) as tc:
        with (
            tc.tile_pool(name="consts", bufs=1) as consts,
            tc.tile_pool(name="big", bufs=1) as big,
            tc.tile_pool(name="xtp", bufs=4) as xtp,
            tc.tile_pool(name="fpool", bufs=4) as fpool,
            tc.tile_pool(name="small", bufs=4) as small,
            tc.tile_pool(name="rrp", bufs=2) as rrp,
            tc.tile_pool(name="ptp", bufs=2) as ptp,
            tc.tile_pool(name="yp", bufs=4) as yp,
            tc.tile_pool(name="ps_s", bufs=2, space="PSUM") as ps_s,
            tc.tile_pool(name="ps_ot", bufs=2, space="PSUM") as ps_ot,
            tc.tile_pool(name="dram", bufs=1, space="DRAM") as dram,
        ):
            # ---- constants ----
            wk_sb = consts.tile([128, 8, DC], BF16)
            nc.sync.dma_start(wk_sb[:], wk_v)
            bk_sb = consts.tile([128, 1], F32)
            nc.sync.dma_start(bk_sb[:], bk.ap())
            wp_sb = consts.tile([128, 8, D], BF16)
            nc.sync.dma_start(wp_sb[:], wp_v)
            bp_rep = consts.tile([128, D], F32)
            nc.sync.dma_start(bp_rep[:], bp.ap().partition_broadcast(128))
            ident = consts.tile([128, 128], F32)
            make_identity(nc, ident[:])

            # persistent activations
            KT = big.tile([128, B, L], F32R)  # [head-dims(2 heads), b, tokens]
            KN = big.tile([128, B, KB, 128], BF16)  # K natural: [tok%128, b, kb, hd]
            OT = big.tile([128, B, L], F32)  # attn out^T (unnormalized, then normed)
            sacc = big.tile([128, B, HPC, KB, L // QS], F32)  # exp row-sum partials

            rs_d = dram.tile([B, HPC, 128, KB], F32)  # 1/s in [tok%128, kb] layout
            cc_in = [dram.tile([N_CORES * 128, TPB], F32, name=f"cc_in{b}") for b in range(B)]
            cc_out = [dram.tile([N_CORES * 128, TPB], F32, name=f"cc_out{b}") for b in range(B)]

            for b in range(B):
                # ---- k-projection: KT[:, b, :] = (x_b @ Wk).T + bk ----
                for ncx in range(L // 512):
                    kp_ps = ps_s.tile([128, QS], F32, tag="s")
                    for dc in range(8):
                        xt_t = xtp.tile([128, 512], BF16)
                        nc.sync.dma_start(
                            xt_t[:], xt_v[b, dc, :, ncx * 512 : (ncx + 1) * 512]
                        )
                        nc.tensor.matmul(
                            kp_ps[:, :512],
                            lhsT=wk_sb[:, dc, :],
                            rhs=xt_t[:],
                            start=(dc == 0),
                            stop=(dc == 7),
                        )
                    nc.vector.tensor_scalar_add(
                        KT[:, b, ncx * 512 : (ncx + 1) * 512], kp_ps[:, :512], bk_sb[:]
                    )

                # ---- K natural layout via PE transposes ----
                for tcx in range(KB):
                    tp_ps = ps_s.tile([128, QS], F32, tag="s")
                    nc.tensor.transpose(
                        tp_ps[:, :128],
                        KT[:, b, tcx * 128 : (tcx + 1) * 128].bitcast(F32),
                        ident[:],
                    )
                    nc.vector.tensor_copy(KN[:, b, tcx, :], tp_ps[:, :128])

                # ---- attention ----
                for qs_i in range(L // QS):
                    q0 = qs_i * QS
                    ot_ps = ps_ot.tile([128, QS], F32, tag="ot")
                    for kb in range(KB):
                        for h in range(HPC):
                            hp = 64 * h
                            s_ps = ps_s.tile([128, QS], F32, tag="s")
                            for qc in range(QS // 512):
                                nc.tensor.matmul(
                                    s_ps[:, qc * 512 : (qc + 1) * 512],
                                    lhsT=KT[hp : hp + 64, b, kb * 128 : (kb + 1) * 128],
                                    rhs=KT[hp : hp + 64, b, q0 + qc * 512 : q0 + (qc + 1) * 512],
                                    start=True,
                                    stop=True,
                                    tile_position=(hp, 0),
                                )
                            f_t = fpool.tile([128, QS], BF16)
                            nc.scalar.activation(
                                f_t[:],
                                s_ps[:],
                                mybir.ActivationFunctionType.Exp,
                                scale=0.125,
                                accum_out=sacc[:, b, h, kb, qs_i : qs_i + 1],
                            )
                            for qc in range(QS // 512):
                                nc.tensor.matmul(
                                    ot_ps[hp : hp + 64, qc * 512 : (qc + 1) * 512],
                                    lhsT=KN[:, b, kb, hp : hp + 64],
                                    rhs=f_t[:, qc * 512 : (qc + 1) * 512],
                                    start=(kb == 0),
                                    stop=(kb == KB - 1),
                                    tile_position=(0, hp),
                                )
                    nc.vector.tensor_copy(OT[:, b, q0 : q0 + QS], ot_ps[:])

                # ---- softmax denominators -> 1/s, replicated across partitions ----
                rr = rrp.tile([128, L], F32)
                for h in range(HPC):
                    s_t = small.tile([128, KB], F32)
                    nc.vector.tensor_add(
                        s_t[:], sacc[:, b, h, :, 0], sacc[:, b, h, :, 1]
                    )
                    rs_t = small.tile([128, KB], F32)
                    nc.vector.reciprocal(rs_t[:], s_t[:])
                    nc.sync.dma_start(rs_d[b, h, :, :], rs_t[:])
                    # token t = kb*128 + p  ->  rs_d[b, h, p, kb]; broadcast to 64 rows
                    for kb in range(KB):
                        nc.sync.dma_start(
                            rr[64 * h : 64 * (h + 1), kb * 128 : (kb + 1) * 128],
                            rs_d[b, h, :, kb].partition_broadcast(64),
                        )
                nc.vector.tensor_mul(OT[:, b, :], OT[:, b, :], rr[:])

                # ---- AllToAll: head-dim shards -> token shards ----
                for j in range(N_CORES):
                    nc.sync.dma_start(
                        cc_in[b][j * 128 : (j + 1) * 128, :],
                        OT[:, b, j * TPB : (j + 1) * TPB],
                    )
                nc.gpsimd.collective_compute(
                    "AllToAll",
                    mybir.AluOpType.bypass,
                    replica_groups=[list(range(N_CORES))],
                    ins=[cc_in[b].opt()],
                    outs=[cc_out[b].opt()],
                )
                pt_f = ptp.tile([128, 8, TPB], F32)
                nc.sync.dma_start(
                    pt_f[:], cc_out[b].rearrange("(po p) t -> p po t", p=128)
                )
                pt = ptp.tile([128, 8, TPB], BF16)
                nc.vector.tensor_copy(pt[:], pt_f[:])

                # ---- output projection for this core's 256-token block ----
                for mc in range(TPB // 128):
                    for nc2 in range(D // 512):
                        pj_ps = ps_s.tile([128, QS], F32, tag="s")
                        for dc in range(8):
                            nc.tensor.matmul(
                                pj_ps[:, :512],
                                lhsT=pt[:, dc, mc * 128 : (mc + 1) * 128],
                                rhs=wp_sb[:, dc, nc2 * 512 : (nc2 + 1) * 512],
                                start=(dc == 0),
                                stop=(dc == 7),
                            )
                        y_t = yp.tile([128, 512], F32)
                        nc.vector.tensor_add(
                            y_t[:], pj_ps[:, :512], bp_rep[:, nc2 * 512 : (nc2 + 1) * 512]
                        )
                        nc.sync.dma_start(
                            out.ap()[b, mc * 128 : (mc + 1) * 128, nc2 * 512 : (nc2 + 1) * 512],
                            y_t[:],
                        )

    nc.compile()
    return nc


_CACHED = None


def _get_nc():
    global _CACHED
    if _CACHED is None:
        _CACHED = build()
    return _CACHED


def run(inputs, trace=False):
    x = np.asarray(inputs["x"], np.float32)
    W_attn = np.asarray(inputs["W_attn"], np.float32)
    b_attn = np.asarray(inputs["b_attn"], np.float32)
    W_proj = np.asarray(inputs["W_proj"], np.float32)
    b_proj = np.asarray(inputs["b_proj"], np.float32)

    import ml_dtypes

    bf16 = ml_dtypes.bfloat16
    xt = np.ascontiguousarray(x.transpose(0, 2, 1)).astype(bf16)  # [B, D, L]
    in_maps = []
    for i in range(N_CORES):
        c0 = D + i * DC
        in_maps.append(
            {
                "xt": xt,
                "wk": np.ascontiguousarray(W_attn[:, c0 : c0 + DC]).astype(bf16),
                "bk": np.ascontiguousarray(b_attn[c0 : c0 + DC].reshape(DC, 1)),
                "wp": W_proj.astype(bf16),
                "bp": b_proj,
            }
        )

    nc = _get_nc()
    res = run_bass_kernel_spmd(
        nc, in_maps, core_ids=list(range(N_CORES)), trace=trace
    )
    outs = np.stack([res.results[i]["out"] for i in range(N_CORES)])  # [8, B, TPB, D]
    y = outs.transpose(1, 0, 2, 3).reshape(B, L, D)
    return y, res


def kernel(**inputs) -> np.ndarray:
    y, _ = run(inputs)
    return y


# revision 6
# speedup vs baseline: 3.0989x; 3.0989x over previous
"""Distributed Bass kernel for nn_Attention (dense transformer block with the
q=k=v source bug) on 8 TRN2 NeuronCores.

Sharding: tensor-parallel over heads (Megatron-style). Core i owns heads
(2i, 2i+1):
  1. k-projection: KT[d, t] = (x @ W_k_slice).T computed from host-transposed
     x^T so no on-device transpose of x is needed. Only the k third of W_attn
     matters (reference takes q = k = v from the k slice).
  2. Scores S = K K^T are SYMMETRIC (q == k), so tiles are produced in
     [key-block, query] layout directly; exp's accum_out gives the softmax
     denominators (row sums == column sums) for free.
  3. O^T = K^T @ E accumulated in PSUM (col-packed: 2 heads side by side).
  4. Normalize by 1/s(q), AllToAll so core i ends with all 1024 head-dims for
     its 256-token block per batch, then the output projection + bias.
Host gathers the 8 per-core [2, 256, 1024] row blocks into [2, 2048, 1024].

All matmuls run in bf16 (f32r streams at 4 cyc/row on HW and rejects column
tile_position); PSUM accumulation is f32. DMAs are coalesced and split across
the SWDGE (gpsimd) and HWDGE (sync) rings — HWDGE DMAs serialize FIFO per
ring at ~2.9us each, so count matters more than bytes.
"""

import numpy as np

import concourse.bass as bass
import concourse.tile as tile
from concourse import bacc, mybir
from concourse.bass_utils import run_bass_kernel_spmd
from concourse.masks import make_identity

N_CORES = 8
B, L, D = 2, 2048, 1024
H, HD = 16, 64
HPC = H // N_CORES  # heads per core = 2
DC = HPC * HD  # head-dim columns per core = 128
TPB = L // N_CORES  # tokens per batch per core (proj stage) = 256
F32 = mybir.dt.float32
BF16 = mybir.dt.bfloat16
QS = 1024  # query span per S-tile (PSUM budget: 2 banks)
KB = L // 128  # 16 key blocks per batch


def build():
    nc = bacc.Bacc("TRN2", target_bir_lowering=False, debug=False, num_devices=N_CORES)
    xt = nc.dram_tensor("xt", [B, D, L], BF16, kind="ExternalInput")
    wk = nc.dram_tensor("wk", [D, DC], BF16, kind="ExternalInput")
    bk = nc.dram_tensor("bk", [DC, 1], F32, kind="ExternalInput")
    wp = nc.dram_tensor("wp", [D, D], BF16, kind="ExternalInput")
    bp = nc.dram_tensor("bp", [D], F32, kind="ExternalInput")
    out = nc.dram_tensor("out", [B, TPB, D], F32, kind="ExternalOutput")

    xt_v = xt.ap().rearrange("b (dc p) t -> b dc p t", p=128)  # [B, 8, 128, L]
    wk_v = wk.ap().rearrange("(dc p) m -> p dc m", p=128)  # [128, 8, DC]
    wp_v = wp.ap().rearrange("(dc p) n -> p dc n", p=128)  # [128, 8, D]

    with tile.TileContext(nc) as tc:
        with (
            tc.tile_pool(name="consts", bufs=1) as consts,
            tc.tile_pool(name="big", bufs=1) as big,
            tc.tile_pool(name="xtp", bufs=1) as xtp,
            tc.tile_pool(name="fpool", bufs=3) as fpool,
            tc.tile_pool(name="small", bufs=4) as small,
            tc.tile_pool(name="rrp", bufs=2) as rrp,
            tc.tile_pool(name="ptp", bufs=2) as ptp,
            tc.tile_pool(name="yp", bufs=2) as yp,
            tc.tile_pool(name="ps_s", bufs=2, space="PSUM") as ps_s,
            tc.tile_pool(name="ps_ot", bufs=1, space="PSUM") as ps_ot,
            tc.tile_pool(name="ps_k", bufs=2, space="PSUM") as ps_k,
            tc.tile_pool(name="dram", bufs=1, space="DRAM") as dram,
        ):
            # ---- constants ----
            wk_sb = consts.tile([128, 8, DC], BF16)
            nc.sync.dma_start(wk_sb[:], wk_v)
            bk_sb = consts.tile([128, 1], F32)
            nc.sync.dma_start(bk_sb[:], bk.ap())
            wp_sb = consts.tile([128, 8, D], BF16)
            nc.sync.dma_start(wp_sb[:], wp_v)
            bp_rep = consts.tile([128, D], F32)
            nc.sync.dma_start(bp_rep[:], bp.ap().partition_broadcast(128))
            identb = consts.tile([128, 128], BF16)
            make_identity(nc, identb[:])
            identf = consts.tile([128, 128], F32)
            make_identity(nc, identf[:])

            # persistent activations
            KT = big.tile([128, B, L], BF16)  # [head-dims(2 heads), b, tokens]
            KN = big.tile([128, B, KB, 128], BF16)  # K natural: [tok%128, b, kb, hd]
            OTn = big.tile([128, B, L], BF16)  # attn out^T; normalized in place
            sacc = big.tile([128, B, HPC, KB, L // QS], F32)  # exp row-sum partials

            rsf_d = dram.tile([B, HPC, KB, 128], BF16)  # 1/s, token-major flat
            cc_in = [dram.tile([N_CORES * 128, TPB], BF16, name=f"cc_in{b}") for b in range(B)]
            cc_out = [dram.tile([N_CORES * 128, TPB], BF16, name=f"cc_out{b}") for b in range(B)]

            XTs = [xtp.tile([128, 8, L], BF16, name=f"xts{b}", tag="xt") for b in range(B)]

            def load_xt(b):
                for dc in range(8):
                    nc.gpsimd.dma_start(XTs[b][:, dc, :], xt_v[b, dc, :, :])

            def kproj(b):
                for ncx in range(L // 512):
                    kp_ps = ps_k.tile([128, 512], F32, tag="k")
                    for dc in range(8):
                        nc.tensor.matmul(
                            kp_ps[:],
                            lhsT=wk_sb[:, dc, :],
                            rhs=XTs[b][:, dc, ncx * 512 : (ncx + 1) * 512],
                            start=(dc == 0),
                            stop=(dc == 7),
                        )
                    nc.vector.tensor_scalar_add(
                        KT[:, b, ncx * 512 : (ncx + 1) * 512], kp_ps[:], bk_sb[:]
                    )
                for tcx in range(KB):
                    tp_ps = ps_k.tile([128, 512], BF16, tag="k")
                    nc.tensor.transpose(
                        tp_ps[:, :128], KT[:, b, tcx * 128 : (tcx + 1) * 128], identb[:]
                    )
                    nc.vector.tensor_copy(KN[:, b, tcx, :], tp_ps[:, :128])

            def attention(b):
                for qs_i in range(L // QS):
                    q0 = qs_i * QS
                    ot_ps = ps_ot.tile([128, QS], F32, tag="ot")
                    for kb in range(KB):
                        for h in range(HPC):
                            hp = 64 * h
                            s_ps = ps_s.tile([128, QS], F32, tag="s")
                            for qc in range(QS // 512):
                                nc.tensor.matmul(
                                    s_ps[:, qc * 512 : (qc + 1) * 512],
                                    lhsT=KT[hp : hp + 64, b, kb * 128 : (kb + 1) * 128],
                                    rhs=KT[hp : hp + 64, b, q0 + qc * 512 : q0 + (qc + 1) * 512],
                                    start=True,
                                    stop=True,
                                    tile_position=(hp, 0),
                                )
                            f_t = fpool.tile([128, QS], BF16, tag="f")
                            nc.scalar.activation(
                                f_t[:],
                                s_ps[:],
                                mybir.ActivationFunctionType.Exp,
                                scale=0.125,
                                accum_out=sacc[:, b, h, kb, qs_i : qs_i + 1],
                            )
                            for qc in range(QS // 512):
                                nc.tensor.matmul(
                                    ot_ps[hp : hp + 64, qc * 512 : (qc + 1) * 512],
                                    lhsT=KN[:, b, kb, hp : hp + 64],
                                    rhs=f_t[:, qc * 512 : (qc + 1) * 512],
                                    start=(kb == 0),
                                    stop=(kb == KB - 1),
                                    tile_position=(0, hp),
                                )
                    nc.vector.tensor_copy(OTn[:, b, q0 : q0 + QS], ot_ps[:])

            def normalize_and_comm(b):
                # 1/s -> token-major DRAM -> partition-broadcast into rr
                rr = rrp.tile([128, L], BF16, tag="rr")
                for h in range(HPC):
                    s_t = small.tile([128, KB], F32, tag="s1")
                    nc.vector.tensor_add(
                        s_t[:], sacc[:, b, h, :, 0], sacc[:, b, h, :, 1]
                    )
                    rs_t = small.tile([128, KB], F32, tag="s2")
                    nc.vector.reciprocal(rs_t[:], s_t[:])
                    rst_ps = ps_k.tile([128, 512], F32, tag="k")
                    nc.tensor.transpose(rst_ps[:KB, :128], rs_t[:], identf[:])
                    rsT = small.tile([KB, 128], BF16, tag="s3")
                    nc.vector.tensor_copy(rsT[:], rst_ps[:KB, :128])
                    nc.sync.dma_start(rsf_d[b, h, :, :], rsT[:])
                    nc.sync.dma_start(
                        rr[64 * h : 64 * (h + 1), :],
                        rsf_d[b, h, :, :].rearrange("a c -> (a c)").partition_broadcast(64),
                    )
                nc.vector.tensor_mul(OTn[:, b, :], OTn[:, b, :], rr[:])
                # one strided DMA builds the shard-major AllToAll input
                nc.sync.dma_start(
                    cc_in[b].rearrange("(j p) t -> p j t", p=128),
                    OTn[:, b, :].rearrange("p (j t) -> p j t", j=N_CORES),
                )
                nc.gpsimd.collective_compute(
                    "AllToAll",
                    mybir.AluOpType.bypass,
                    replica_groups=[list(range(N_CORES))],
                    ins=[cc_in[b].opt()],
                    outs=[cc_out[b].opt()],
                )

            def proj(b):
                pt = ptp.tile([128, 8, TPB], BF16, tag="pt")
                nc.gpsimd.dma_start(
                    pt[:], cc_out[b].rearrange("(po p) t -> p po t", p=128)
                )
                for mc in range(TPB // 128):
                    y_t = yp.tile([128, D], F32, tag="y")
                    for nc2 in range(D // 512):
                        pj_ps = ps_k.tile([128, 512], F32, tag="k")
                        for dc in range(8):
                            nc.tensor.matmul(
                                pj_ps[:],
                                lhsT=pt[:, dc, mc * 128 : (mc + 1) * 128],
                                rhs=wp_sb[:, dc, nc2 * 512 : (nc2 + 1) * 512],
                                start=(dc == 0),
                                stop=(dc == 7),
                            )
                        nc.vector.tensor_add(
                            y_t[:, nc2 * 512 : (nc2 + 1) * 512],
                            pj_ps[:],
                            bp_rep[:, nc2 * 512 : (nc2 + 1) * 512],
                        )
                    nc.sync.dma_start(
                        out.ap()[b, mc * 128 : (mc + 1) * 128, :], y_t[:]
                    )

            # ---- schedule (program order = scheduler priority) ----
            load_xt(0)
            kproj(0)
            attention(0)
            load_xt(1)  # prefetch ahead of the b0 collective
            kproj(1)
            normalize_and_comm(0)
            attention(1)
            proj(0)
            normalize_and_comm(1)
            proj(1)

    nc.compile()
    return nc


_CACHED = None


def _get_nc():
    global _CACHED
    if _CACHED is None:
        _CACHED = build()
    return _CACHED


def run(inputs, trace=False):
    import ml_dtypes

    bf16 = ml_dtypes.bfloat16
    x = np.asarray(inputs["x"], np.float32)
    W_attn = np.asarray(inputs["W_attn"], np.float32)
    b_attn = np.asarray(inputs["b_attn"], np.float32)
    W_proj = np.asarray(inputs["W_proj"], np.float32)
    b_proj = np.asarray(inputs["b_proj"], np.float32)

    xt = np.ascontiguousarray(x.transpose(0, 2, 1)).astype(bf16)  # [B, D, L]
    wp16 = W_proj.astype(bf16)
    in_maps = []
    for i in range(N_CORES):
        c0 = D + i * DC
        in_maps.append(
            {
                "xt": xt,
                "wk": np.ascontiguousarray(W_attn[:, c0 : c0 + DC]).astype(bf16),
                "bk": np.ascontiguousarray(b_attn[c0 : c0 + DC].reshape(DC, 1)),
                "wp": wp16,
                "bp": b_proj,
            }
        )

    nc = _get_nc()
    res = run_bass_kernel_spmd(
        nc, in_maps, core_ids=list(range(N_CORES)), trace=trace
    )
    outs = np.stack([res.results[i]["out"] for i in range(N_CORES)])  # [8, B, TPB, D]
    y = outs.transpose(1, 0, 2, 3).reshape(B, L, D)
    return y, res


def kernel(**inputs) -> np.ndarray:
    y, _ = run(inputs)
    return y


# revision 7
# speedup vs baseline: 3.2418x; 1.0461x over previous
"""Distributed Bass kernel for nn_Attention (dense transformer block with the
q=k=v source bug) on 8 TRN2 NeuronCores.

Sharding: tensor-parallel over heads (Megatron-style). Core i owns heads
(2i, 2i+1):
  1. k-projection: KT[d, t] = (x @ W_k_slice).T computed from host-transposed
     x^T so no on-device transpose of x is needed. Only the k third of W_attn
     matters (reference takes q = k = v from the k slice).
  2. Scores S = K K^T are SYMMETRIC (q == k), so tiles are produced in
     [key-block, query] layout directly; exp's accum_out gives the softmax
     denominators (row sums == column sums) for free.
  3. O^T = K^T @ E accumulated in PSUM (col-packed: 2 heads side by side).
  4. Normalize by 1/s(q), AllToAll so core i ends with all 1024 head-dims for
     its 256-token block per batch, then the output projection + bias.
Host gathers the 8 per-core [2, 256, 1024] row blocks into [2, 2048, 1024].

All matmuls run in bf16 (f32r streams at 4 cyc/row on HW and rejects column
tile_position); PSUM accumulation is f32. DMAs are coalesced and split across
the SWDGE (gpsimd) and HWDGE (sync) rings — HWDGE DMAs serialize FIFO per
ring at ~2.9us each, so count matters more than bytes.
"""

import numpy as np

import concourse.bass as bass
import concourse.tile as tile
from concourse import bacc, mybir
from concourse.bass_utils import run_bass_kernel_spmd
from concourse.masks import make_identity

N_CORES = 8
B, L, D = 2, 2048, 1024
H, HD = 16, 64
HPC = H // N_CORES  # heads per core = 2
DC = HPC * HD  # head-dim columns per core = 128
TPB = L // N_CORES  # tokens per batch per core (proj stage) = 256
F32 = mybir.dt.float32
BF16 = mybir.dt.bfloat16
QS = 1024  # query span per S-tile (PSUM budget: 2 banks)
KB = L // 128  # 16 key blocks per batch


def build():
    nc = bacc.Bacc("TRN2", target_bir_lowering=False, debug=False, num_devices=N_CORES)
    xt = nc.dram_tensor("xt", [B, D, L], BF16, kind="ExternalInput")
    wk = nc.dram_tensor("wk", [D, DC], BF16, kind="ExternalInput")
    bk = nc.dram_tensor("bk", [DC, 1], F32, kind="ExternalInput")
    wp = nc.dram_tensor("wp", [D, D], BF16, kind="ExternalInput")
    bp = nc.dram_tensor("bp", [D], F32, kind="ExternalInput")
    out = nc.dram_tensor("out", [B, TPB, D], F32, kind="ExternalOutput")

    xt_v = xt.ap().rearrange("b (dc p) t -> b dc p t", p=128)  # [B, 8, 128, L]
    wk_v = wk.ap().rearrange("(dc p) m -> p dc m", p=128)  # [128, 8, DC]
    wp_v = wp.ap().rearrange("(dc p) n -> p dc n", p=128)  # [128, 8, D]

    with tile.TileContext(nc) as tc:
        with (
            tc.tile_pool(name="consts", bufs=1) as consts,
            tc.tile_pool(name="big", bufs=1) as big,
            tc.tile_pool(name="xtp", bufs=1) as xtp,
            tc.tile_pool(name="fpool", bufs=3) as fpool,
            tc.tile_pool(name="small", bufs=4) as small,
            tc.tile_pool(name="rrp", bufs=2) as rrp,
            tc.tile_pool(name="ptp", bufs=2) as ptp,
            tc.tile_pool(name="yp", bufs=2) as yp,
            tc.tile_pool(name="ps_s", bufs=2, space="PSUM") as ps_s,
            tc.tile_pool(name="ps_ot", bufs=1, space="PSUM") as ps_ot,
            tc.tile_pool(name="ps_k", bufs=2, space="PSUM") as ps_k,
            tc.tile_pool(name="dram", bufs=1, space="DRAM") as dram,
        ):
            # ---- constants ----
            wk_sb = consts.tile([128, 8, DC], BF16)
            nc.sync.dma_start(wk_sb[:], wk_v)
            bk_sb = consts.tile([128, 1], F32)
            nc.sync.dma_start(bk_sb[:], bk.ap())
            wp_sb = consts.tile([128, 8, D], BF16)
            nc.sync.dma_start(wp_sb[:], wp_v)
            bp_rep = consts.tile([128, D], F32)
            nc.sync.dma_start(bp_rep[:], bp.ap().partition_broadcast(128))
            identb = consts.tile([128, 128], BF16)
            make_identity(nc, identb[:])
            identf = consts.tile([128, 128], F32)
            make_identity(nc, identf[:])

            # persistent activations — separate tiles per batch (and per 512-token
            # chunk for KT) so cross-batch writes never alias reads
            KTc = [[big.tile([128, 512], BF16, name=f"kt{b}_{c}") for c in range(4)]
                   for b in range(B)]
            KNs = [big.tile([128, KB, 128], BF16, name=f"kn{b}") for b in range(B)]
            OTns = [big.tile([128, L], BF16, name=f"otn{b}") for b in range(B)]
            saccs = [big.tile([128, HPC, KB, L // QS], F32, name=f"sacc{b}") for b in range(B)]

            rsf_d = dram.tile([B, HPC, KB, 128], BF16)  # 1/s, token-major flat
            NH = 2  # token halves per batch for the A2A/proj pipeline
            TH = TPB // NH  # 128 tokens per half
            cc_in = [[dram.tile([N_CORES * 128, TH], BF16, name=f"cc_in{b}_{h}")
                      for h in range(NH)] for b in range(B)]
            cc_out = [[dram.tile([N_CORES * 128, TH], BF16, name=f"cc_out{b}_{h}")
                       for h in range(NH)] for b in range(B)]

            XTs = [[xtp.tile([128, 8, 512], BF16, name=f"xts{b}_{c}", tag=f"xt{c}")
                    for c in range(4)] for b in range(B)]

            def load_xt(b):
                for c in range(4):
                    for dc in range(8):
                        nc.gpsimd.dma_start(
                            XTs[b][c][:, dc, :], xt_v[b, dc, :, c * 512 : (c + 1) * 512]
                        )

            def kproj(b):
                for ncx in range(L // 512):
                    kp_ps = ps_k.tile([128, 512], F32, tag="k")
                    for dc in range(8):
                        nc.tensor.matmul(
                            kp_ps[:],
                            lhsT=wk_sb[:, dc, :],
                            rhs=XTs[b][ncx][:, dc, :],
                            start=(dc == 0),
                            stop=(dc == 7),
                        )
                    nc.vector.tensor_scalar_add(KTc[b][ncx][:], kp_ps[:], bk_sb[:])
                    for sub in range(4):
                        tcx = ncx * 4 + sub
                        tp_ps = ps_k.tile([128, 512], BF16, tag="k")
                        nc.tensor.transpose(
                            tp_ps[:, :128],
                            KTc[b][ncx][:, sub * 128 : (sub + 1) * 128],
                            identb[:],
                        )
                        nc.vector.tensor_copy(KNs[b][:, tcx, :], tp_ps[:, :128])

            def attention(b):
                for qs_i in range(L // QS):
                    q0 = qs_i * QS
                    ot_ps = ps_ot.tile([128, QS], F32, tag="ot")
                    for kb in range(KB):
                        for h in range(HPC):
                            hp = 64 * h
                            s_ps = ps_s.tile([128, QS], F32, tag="s")
                            for qc in range(QS // 512):
                                nc.tensor.matmul(
                                    s_ps[:, qc * 512 : (qc + 1) * 512],
                                    lhsT=KTc[b][kb // 4][hp : hp + 64, (kb % 4) * 128 : (kb % 4 + 1) * 128],
                                    rhs=KTc[b][qs_i * 2 + qc][hp : hp + 64, :],
                                    start=True,
                                    stop=True,
                                    tile_position=(hp, 0),
                                )
                            f_t = fpool.tile([128, QS], BF16, tag="f")
                            nc.scalar.activation(
                                f_t[:],
                                s_ps[:],
                                mybir.ActivationFunctionType.Exp,
                                scale=0.125,
                                accum_out=saccs[b][:, h, kb, qs_i : qs_i + 1],
                            )
                            for qc in range(QS // 512):
                                nc.tensor.matmul(
                                    ot_ps[hp : hp + 64, qc * 512 : (qc + 1) * 512],
                                    lhsT=KNs[b][:, kb, hp : hp + 64],
                                    rhs=f_t[:, qc * 512 : (qc + 1) * 512],
                                    start=(kb == 0),
                                    stop=(kb == KB - 1),
                                    tile_position=(0, hp),
                                )
                    nc.vector.tensor_copy(OTns[b][:, q0 : q0 + QS], ot_ps[:])

            def normalize(b):
                # 1/s -> token-major DRAM -> partition-broadcast into rr
                rr = rrp.tile([128, L], BF16, tag="rr")
                for h in range(HPC):
                    s_t = small.tile([128, KB], F32, tag="s1")
                    nc.vector.tensor_add(
                        s_t[:], saccs[b][:, h, :, 0], saccs[b][:, h, :, 1]
                    )
                    rs_t = small.tile([128, KB], F32, tag="s2")
                    nc.vector.reciprocal(rs_t[:], s_t[:])
                    rst_ps = ps_k.tile([128, 512], F32, tag="k")
                    nc.tensor.transpose(rst_ps[:KB, :128], rs_t[:], identf[:])
                    rsT = small.tile([KB, 128], BF16, tag="s3")
                    nc.vector.tensor_copy(rsT[:], rst_ps[:KB, :128])
                    nc.sync.dma_start(rsf_d[b, h, :, :], rsT[:])
                    nc.sync.dma_start(
                        rr[64 * h : 64 * (h + 1), :],
                        rsf_d[b, h, :, :].rearrange("a c -> (a c)").partition_broadcast(64),
                    )
                nc.vector.tensor_mul(OTns[b][:], OTns[b][:], rr[:])

            def comm(b, hf):
                nc.sync.dma_start(
                    cc_in[b][hf].rearrange("(j p) t -> p j t", p=128),
                    OTns[b].rearrange("p (j u t) -> p j u t", j=N_CORES, u=NH)[:, :, hf, :],
                )
                nc.gpsimd.collective_compute(
                    "AllToAll",
                    mybir.AluOpType.bypass,
                    replica_groups=[list(range(N_CORES))],
                    ins=[cc_in[b][hf].opt()],
                    outs=[cc_out[b][hf].opt()],
                )

            def proj(b, hf):
                pt = ptp.tile([128, 8, TH], BF16, tag="pt")
                nc.gpsimd.dma_start(
                    pt[:], cc_out[b][hf].rearrange("(po p) t -> p po t", p=128)
                )
                y_t = yp.tile([128, D], F32, tag="y")
                for nc2 in range(D // 512):
                    pj_ps = ps_k.tile([128, 512], F32, tag="k")
                    for dc in range(8):
                        nc.tensor.matmul(
                            pj_ps[:],
                            lhsT=pt[:, dc, :],
                            rhs=wp_sb[:, dc, nc2 * 512 : (nc2 + 1) * 512],
                            start=(dc == 0),
                            stop=(dc == 7),
                        )
                    nc.vector.tensor_add(
                        y_t[:, nc2 * 512 : (nc2 + 1) * 512],
                        pj_ps[:],
                        bp_rep[:, nc2 * 512 : (nc2 + 1) * 512],
                    )
                nc.sync.dma_start(
                    out.ap()[b, hf * TH : (hf + 1) * TH, :], y_t[:]
                )

            # ---- schedule (program order = scheduler priority) ----
            load_xt(0)
            kproj(0)
            attention(0)
            load_xt(1)  # prefetch ahead of the b0 collective
            kproj(1)
            normalize(0)
            comm(0, 0)
            comm(0, 1)
            proj(0, 0)
            proj(0, 1)
            attention(1)
            normalize(1)
            comm(1, 0)
            proj(1, 0)
            comm(1, 1)
            proj(1, 1)

    nc.compile()
    return nc


_CACHED = None


def _get_nc():
    global _CACHED
    if _CACHED is None:
        _CACHED = build()
    return _CACHED


def run(inputs, trace=False):
    import ml_dtypes

    bf16 = ml_dtypes.bfloat16
    x = np.asarray(inputs["x"], np.float32)
    W_attn = np.asarray(inputs["W_attn"], np.float32)
    b_attn = np.asarray(inputs["b_attn"], np.float32)
    W_proj = np.asarray(inputs["W_proj"], np.float32)
    b_proj = np.asarray(inputs["b_proj"], np.float32)

    xt = np.ascontiguousarray(x.transpose(0, 2, 1)).astype(bf16)  # [B, D, L]
    wp16 = W_proj.astype(bf16)
    in_maps = []
    for i in range(N_CORES):
        c0 = D + i * DC
        in_maps.append(
            {
                "xt": xt,
                "wk": np.ascontiguousarray(W_attn[:, c0 : c0 + DC]).astype(bf16),
                "bk": np.ascontiguousarray(b_attn[c0 : c0 + DC].reshape(DC, 1)),
                "wp": wp16,
                "bp": b_proj,
            }
        )

    nc = _get_nc()
    res = run_bass_kernel_spmd(
        nc, in_maps, core_ids=list(range(N_CORES)), trace=trace
    )
    outs = np.stack([res.results[i]["out"] for i in range(N_CORES)])  # [8, B, TPB, D]
    y = outs.transpose(1, 0, 2, 3).reshape(B, L, D)
    return y, res


def kernel(**inputs) -> np.ndarray:
    y, _ = run(inputs)
    return y
